# revision 6
# baseline (speedup 1.0000x reference)
"""DAGCN layer kernel for 8 Trainium2 NeuronCores (Bass/Tile, SPMD).

Math (equal to the reference by linearity of the edge MLP):
  hs = h @ W_src ; ht = h @ W_tgt
  agg[n] = (sum_{e:dst=n} hs[src[e]] + deg[n]*(ht[n] + b_src + b_tgt)) / max(deg[n],1)
  then multi-head attention (q from h rows, k/v from agg) + FFN with LayerNorms.

Sharding: edges sorted by dst, bucketed per (core, 128-node block), padded to a
fixed tile count; core c owns dst/query rows [512c, 512c+512). Per-edge work is
an indirect-DMA gather of hs-table rows + one-hot matmul scatter into PSUM.
agg slices are AllGathered (bf16, transposed) so each core holds full k/v.
"""

import contextlib
import numpy as np
import ml_dtypes

import concourse.bass as bass
import concourse.bacc as bacc
import concourse.tile as tile
from concourse import mybir
from concourse.bass_utils import run_bass_kernel_spmd
from concourse.masks import make_identity

N, H, HEADS, E = 4096, 256, 4, 262144
DH = H // HEADS          # 64
NCORES = 8
RPC = N // NCORES        # 512 nodes per core
NBLK = RPC // 128        # 4 dst blocks per core
TPB = 68                 # tiles per block (68*128 = 8704 padded edges per block)
NT = NBLK * TPB          # 272 edge tiles per core
D = H + 2                # table row: 256 features + ones col + pad
TBL_ROWS = N + 128
EPS = 1e-5
NCH = N // 128           # 32
HCH = H // 128           # 2

BF = mybir.dt.bfloat16
F32 = mybir.dt.float32
I32 = mybir.dt.int32
AF = mybir.ActivationFunctionType

_CACHE = {}


def _layernorm_rows(nc, pool, z, out_t, g, be, i, nm, epst=None):
    """LayerNorm along the free dim of a [128, H] f32 row tile."""
    stats = pool.tile([128, 6], F32, name=f"{nm}_st{i}", tag=f"{nm}_st")
    nc.vector.bn_stats(out=stats[:], in_=z[:])
    mv = pool.tile([128, 2], F32, name=f"{nm}_mv{i}", tag=f"{nm}_mv")
    nc.vector.bn_aggr(out=mv[:], in_=stats[:])
    sd = pool.tile([128, 1], F32, name=f"{nm}_sd{i}", tag=f"{nm}_sd")
    nc.scalar.activation(out=sd[:], in_=mv[:, 1:2], func=AF.Sqrt,
                         bias=epst[:, 0:1], scale=1.0)
    rstd = pool.tile([128, 1], F32, name=f"{nm}_rs{i}", tag=f"{nm}_rs")
    nc.vector.reciprocal(out=rstd[:], in_=sd[:])
    nmu = pool.tile([128, 1], F32, name=f"{nm}_nm{i}", tag=f"{nm}_nm")
    nc.vector.tensor_tensor(out=nmu[:], in0=mv[:, 0:1], in1=rstd[:],
                            op=mybir.AluOpType.mult)
    nc.vector.tensor_scalar_mul(nmu[:], nmu[:], -1.0)
    zn = pool.tile([128, z.shape[1]], F32, name=f"{nm}_zn{i}", tag=f"{nm}_zn")
    nc.scalar.activation(out=zn[:], in_=z[:], func=AF.Identity,
                         bias=nmu[:, 0:1], scale=rstd[:, 0:1])
    nc.vector.tensor_tensor(out=zn[:], in0=zn[:], in1=g[:], op=mybir.AluOpType.mult)
    nc.vector.tensor_add(out_t[:], zn[:], be[:])


def _build_program():
    nc = bacc.Bacc("TRN2", target_bir_lowering=False, debug=False, num_devices=NCORES)

    hT_bf = nc.dram_tensor("hT_bf", [H, N], BF, kind="ExternalInput")
    hT_own_d = nc.dram_tensor("hT_own", [H, RPC], BF, kind="ExternalInput")
    h_rows = nc.dram_tensor("h_rows", [RPC, H], F32, kind="ExternalInput")
    srcT = nc.dram_tensor("srcT", [128, NT], I32, kind="ExternalInput")
    dstT = nc.dram_tensor("dstT", [128, NT], I32, kind="ExternalInput")
    w_src = nc.dram_tensor("w_src", [H, H], BF, kind="ExternalInput")
    w_tgt = nc.dram_tensor("w_tgt", [H, H], BF, kind="ExternalInput")
    w_q = nc.dram_tensor("w_q", [H, H], BF, kind="ExternalInput")
    w_k = nc.dram_tensor("w_k", [H, H], BF, kind="ExternalInput")
    w_v = nc.dram_tensor("w_v", [H, H], BF, kind="ExternalInput")
    w_o = nc.dram_tensor("w_o", [H, H], BF, kind="ExternalInput")
    w_1 = nc.dram_tensor("w_1", [H, 2 * H], BF, kind="ExternalInput")
    w_2 = nc.dram_tensor("w_2", [2 * H, H], BF, kind="ExternalInput")
    bst_b = nc.dram_tensor("bst_b", [128, H], F32, kind="ExternalInput")
    bq_c = nc.dram_tensor("bq_c", [128, HCH], F32, kind="ExternalInput")
    bk_c = nc.dram_tensor("bk_c", [128, HCH], F32, kind="ExternalInput")
    bv_b = nc.dram_tensor("bv_b", [128, H], F32, kind="ExternalInput")
    bo_b = nc.dram_tensor("bo_b", [128, H], F32, kind="ExternalInput")
    b1_c = nc.dram_tensor("b1_c", [128, 4], F32, kind="ExternalInput")
    b2_b = nc.dram_tensor("b2_b", [128, H], F32, kind="ExternalInput")
    g1_b = nc.dram_tensor("g1_b", [128, H], F32, kind="ExternalInput")
    be1_b = nc.dram_tensor("be1_b", [128, H], F32, kind="ExternalInput")
    g2_b = nc.dram_tensor("g2_b", [128, H], F32, kind="ExternalInput")
    be2_b = nc.dram_tensor("be2_b", [128, H], F32, kind="ExternalInput")
    out = nc.dram_tensor("out", [RPC, H], F32, kind="ExternalOutput")
    table = nc.dram_tensor("hs_table", [TBL_ROWS, D], BF)

    with tile.TileContext(nc) as tc, contextlib.ExitStack() as ctx:
        singles = ctx.enter_context(tc.tile_pool(name="singles", bufs=1))
        wpool = ctx.enter_context(tc.tile_pool(name="wpool", bufs=1))
        hs_sb = ctx.enter_context(tc.tile_pool(name="hs_sb", bufs=4))
        gpool = ctx.enter_context(tc.tile_pool(name="gpool", bufs=16))
        ohpool = ctx.enter_context(tc.tile_pool(name="ohpool", bufs=8))
        epool = ctx.enter_context(tc.tile_pool(name="epool", bufs=8))

        # ---------- constants ----------
        hT = [singles.tile([128, N], BF, name=f"hT{j}") for j in range(HCH)]
        for j in range(HCH):
            nc.sync.dma_start(out=hT[j][:], in_=hT_bf[j * 128:(j + 1) * 128, :])
        hTo = [singles.tile([128, RPC], BF, name=f"hTo{j}") for j in range(HCH)]
        for j in range(HCH):
            nc.sync.dma_start(out=hTo[j][:], in_=hT_own_d[j * 128:(j + 1) * 128, :])

        def load_w(t, name, rows, cols):
            w = [wpool.tile([128, cols], BF, name=f"{name}{i}") for i in range(rows // 128)]
            for i in range(rows // 128):
                nc.sync.dma_start(out=w[i][:], in_=t[i * 128:(i + 1) * 128, :])
            return w

        Wsrc = load_w(w_src, "Wsrc", H, H)
        Wtgt = load_w(w_tgt, "Wtgt", H, H)
        Wq = load_w(w_q, "Wq", H, H)
        Wk = load_w(w_k, "Wk", H, H)
        Wv = load_w(w_v, "Wv", H, H)
        Wo = load_w(w_o, "Wo", H, H)
        W1 = load_w(w_1, "W1", H, 2 * H)
        W2 = load_w(w_2, "W2", 2 * H, H)

        def load_b(t, name, shape):
            b = singles.tile(list(shape), F32, name=name)
            nc.sync.dma_start(out=b[:], in_=t[:])
            return b

        bstb = load_b(bst_b, "bstb", (128, H))
        bqc = load_b(bq_c, "bqc", (128, HCH))
        bkc = load_b(bk_c, "bkc", (128, HCH))
        bvb = load_b(bv_b, "bvb", (128, H))
        bob = load_b(bo_b, "bob", (128, H))
        b1c = load_b(b1_c, "b1c", (128, 4))
        b2b = load_b(b2_b, "b2b", (128, H))
        g1b = load_b(g1_b, "g1b", (128, H))
        be1b = load_b(be1_b, "be1b", (128, H))
        g2b = load_b(g2_b, "g2b", (128, H))
        be2b = load_b(be2_b, "be2b", (128, H))

        src_t = singles.tile([128, NT], I32)
        nc.sync.dma_start(out=src_t[:], in_=srcT[:])
        dst_raw = singles.tile([128, NT], I32)
        nc.sync.dma_start(out=dst_raw[:], in_=dstT[:])
        dst_t = singles.tile([128, NT], F32)
        nc.vector.tensor_copy(out=dst_t[:], in_=dst_raw[:])
        iotaf = []
        for b in range(NBLK):
            it_i = singles.tile([128, 128], I32, name=f"iotai{b}")
            nc.gpsimd.iota(it_i[:], pattern=[[1, 128]], base=b * 128, channel_multiplier=0)
            it_f = singles.tile([128, 128], F32, name=f"iotaf{b}")
            nc.vector.tensor_copy(out=it_f[:], in_=it_i[:])
            iotaf.append(it_f)
        hrows = [singles.tile([128, H], F32, name=f"hrows{i}") for i in range(NBLK)]
        for i in range(NBLK):
            nc.sync.dma_start(out=hrows[i][:], in_=h_rows[i * 128:(i + 1) * 128, :])
        ident = singles.tile([128, 128], F32)
        make_identity(nc, ident[:])
        epst = singles.tile([128, 1], F32)
        nc.vector.memset(epst[:], EPS)

        # ---------- phase 1: hs table, ht rows, qT ----------
        ph1 = tc.tile_pool(name="ph1_ps", bufs=2, space="PSUM")
        hs_ps = mid_ps = ph1.__enter__()
        for nch in range(NCH):
            ps = hs_ps.tile([128, H], F32)
            for k in range(HCH):
                nc.tensor.matmul(out=ps[:], lhsT=hT[k][:, nch * 128:(nch + 1) * 128],
                                 rhs=Wsrc[k][:], start=(k == 0), stop=(k == HCH - 1))
            row = hs_sb.tile([128, D], BF)
            nc.scalar.copy(out=row[:, 0:H], in_=ps[:])
            nc.vector.memset(row[:, H:H + 1], 1.0)
            nc.vector.memset(row[:, H + 1:D], 0.0)
            nc.sync.dma_start(out=table[nch * 128:(nch + 1) * 128, :], in_=row[:])
        zrow = singles.tile([128, D], BF)
        nc.vector.memset(zrow[:], 0.0)
        nc.sync.dma_start(out=table[N:N + 128, :], in_=zrow[:])

        htr = [singles.tile([128, H], F32, name=f"htr{i}") for i in range(NBLK)]
        for i in range(NBLK):
            ps = mid_ps.tile([128, H], F32)
            for k in range(HCH):
                nc.tensor.matmul(out=ps[:], lhsT=hTo[k][:, i * 128:(i + 1) * 128],
                                 rhs=Wtgt[k][:], start=(k == 0), stop=(k == HCH - 1))
            nc.vector.tensor_copy(out=htr[i][:], in_=ps[:])

        qT = [singles.tile([128, RPC], BF, name=f"qT{j}") for j in range(HCH)]
        for j in range(HCH):
            ps = mid_ps.tile([128, RPC], F32)
            for k in range(HCH):
                nc.tensor.matmul(out=ps[:], lhsT=Wq[k][:, j * 128:(j + 1) * 128],
                                 rhs=hTo[k][:], start=(k == 0), stop=(k == HCH - 1))
            nc.scalar.activation(out=qT[j][:], in_=ps[:], func=AF.Identity,
                                 bias=bqc[:, j:j + 1], scale=1.0)

        ph1.__exit__(None, None, None)

        # ---------- phase 2: gather + one-hot scatter ----------
        ph2 = tc.tile_pool(name="agg_ps", bufs=1, space="PSUM")
        agg_ps = ph2.__enter__()
        aggp = [agg_ps.tile([128, D], F32, name=f"aggp{b}") for b in range(NBLK)]
        for t in range(NT):
            b = t // TPB
            g = gpool.tile([128, D], BF)
            nc.gpsimd.indirect_dma_start(
                out=g[:], out_offset=None, in_=table[:],
                in_offset=bass.IndirectOffsetOnAxis(ap=src_t[:, t:t + 1], axis=0))
            oh = ohpool.tile([128, 128], BF)
            nc.vector.tensor_tensor(
                out=oh[:], in0=dst_t[:, t:t + 1].to_broadcast([128, 128]),
                in1=iotaf[b][:], op=mybir.AluOpType.is_equal)
            nc.tensor.matmul(out=aggp[b][:], lhsT=oh[:], rhs=g[:],
                             start=(t % TPB == 0), stop=(t % TPB == TPB - 1))

        # finalize agg rows
        agg_rows = [singles.tile([128, H], F32, name=f"aggrow{b}") for b in range(NBLK)]
        for b in range(NBLK):
            deg = singles.tile([128, 1], F32, name=f"deg{b}")
            nc.vector.tensor_copy(out=deg[:], in_=aggp[b][:, H:H + 1])
            tmp = singles.tile([128, H], F32, name=f"fin_tmp{b}", tag="fin_tmp")
            nc.vector.tensor_add(tmp[:], htr[b][:], bstb[:])
            nc.vector.tensor_tensor(out=tmp[:], in0=tmp[:],
                                    in1=deg[:, 0:1].to_broadcast([128, H]),
                                    op=mybir.AluOpType.mult)
            nc.vector.tensor_add(tmp[:], tmp[:], aggp[b][:, 0:H])
            degc = singles.tile([128, 1], F32, name=f"degc{b}", tag="degc")
            nc.vector.tensor_scalar_max(degc[:], deg[:], 1.0)
            rec = singles.tile([128, 1], F32, name=f"rec{b}", tag="recb")
            nc.vector.reciprocal(out=rec[:], in_=degc[:])
            nc.vector.tensor_tensor(out=agg_rows[b][:], in0=tmp[:],
                                    in1=rec[:, 0:1].to_broadcast([128, H]),
                                    op=mybir.AluOpType.mult)

        ph2.__exit__(None, None, None)
        ph3 = tc.tile_pool(name="tp_ps", bufs=2, space="PSUM")
        tp_ps = ph3.__enter__()
        aggT_own = [singles.tile([128, RPC], BF, name=f"aggTo{j}") for j in range(HCH)]
        for b in range(NBLK):
            for j in range(HCH):
                tp = tp_ps.tile([128, 128], F32, name=f"tp_{b}_{j}", tag="tp")
                nc.tensor.transpose(out=tp[:], in_=agg_rows[b][:, j * 128:(j + 1) * 128],
                                    identity=ident[:])
                nc.vector.tensor_copy(out=aggT_own[j][:, b * 128:(b + 1) * 128], in_=tp[:])

        ph3.__exit__(None, None, None)

        # ---------- phase 3: AllGather ----------
        with tc.tile_pool(name="dram", bufs=1, space="DRAM") as dram:
            cc_in = dram.tile([H, RPC], BF)
            cc_out = dram.tile([NCORES * H, RPC], BF, addr_space="Shared")
            for j in range(HCH):
                nc.gpsimd.dma_start(out=cc_in[j * 128:(j + 1) * 128, :], in_=aggT_own[j][:])
            nc.gpsimd.collective_compute(
                "AllGather", mybir.AluOpType.bypass,
                replica_groups=[list(range(NCORES))],
                ins=[cc_in.opt()], outs=[cc_out.opt()])
            aggT_full = [singles.tile([128, N], BF, name=f"aggTf{j}") for j in range(HCH)]
            ccv = cc_out[:].rearrange("(c h) f -> c h f", c=NCORES)
            for j in range(HCH):
                for c in range(NCORES):
                    nc.sync.dma_start(out=aggT_full[j][:, c * RPC:(c + 1) * RPC],
                                      in_=ccv[c, j * 128:(j + 1) * 128, :])

            # ---------- phase 4: kT and v_ext ----------
            ph4 = tc.tile_pool(name="kv_ps", bufs=2, space="PSUM")
            mid_ps = ph4.__enter__()
            kT = [singles.tile([128, N], BF, name=f"kT{j}") for j in range(HCH)]
            for j in range(HCH):
                for piece in range(N // 512):
                    ps = mid_ps.tile([128, 512], F32)
                    for k in range(HCH):
                        nc.tensor.matmul(
                            out=ps[:], lhsT=Wk[k][:, j * 128:(j + 1) * 128],
                            rhs=aggT_full[k][:, piece * 512:(piece + 1) * 512],
                            start=(k == 0), stop=(k == HCH - 1))
                    nc.scalar.activation(out=kT[j][:, piece * 512:(piece + 1) * 512],
                                         in_=ps[:], func=AF.Identity,
                                         bias=bkc[:, j:j + 1], scale=1.0)
            vext = [singles.tile([128, HEADS * (DH + 1)], BF, name=f"vext{kc}")
                    for kc in range(NCH)]
            for kc in range(NCH):
                ps = mid_ps.tile([128, H], F32)
                for k in range(HCH):
                    nc.tensor.matmul(out=ps[:], lhsT=aggT_full[k][:, kc * 128:(kc + 1) * 128],
                                     rhs=Wv[k][:], start=(k == 0), stop=(k == HCH - 1))
                vtmp = singles.tile([128, H], F32, name=f"vtmp{kc}", tag="vtmp")
                nc.vector.tensor_add(vtmp[:], ps[:], bvb[:])
                for h in range(HEADS):
                    nc.vector.tensor_copy(out=vext[kc][:, h * (DH + 1):h * (DH + 1) + DH],
                                          in_=vtmp[:, h * DH:(h + 1) * DH])
                    nc.vector.memset(vext[kc][:, h * (DH + 1) + DH:(h + 1) * (DH + 1)], 1.0)

            ph4.__exit__(None, None, None)

            # ---------- phase 5: attention ----------
            SCALE = float(1.0 / np.sqrt(DH))
            ph5c = tc.tile_pool(name="ctx_ps", bufs=1, space="PSUM")
            ctx_ps = ph5c.__enter__()
            ph5q = tc.tile_pool(name="qk_ps", bufs=2, space="PSUM")
            qk_ps = ph5q.__enter__()
            ctxp = [ctx_ps.tile([DH + 1, RPC], F32, name=f"ctxp{h}") for h in range(HEADS)]
            for kc in range(NCH):
                es = []
                for h in range(HEADS):
                    j, r = h // 2, (h % 2) * 64
                    ps_s = qk_ps.tile([128, RPC], F32, name=f"ps_s{h}_{kc}",
                                      tag=f"ps_s{h % 2}")
                    nc.tensor.matmul(out=ps_s[:],
                                     lhsT=kT[j][r:r + 64, kc * 128:(kc + 1) * 128],
                                     rhs=qT[j][r:r + 64, :], start=True, stop=True,
                                     tile_position=(r, 0))
                    e = epool.tile([128, RPC], BF, name=f"e{h}_{kc}", tag=f"e{h}")
                    nc.scalar.activation(out=e[:], in_=ps_s[:], func=AF.Exp, scale=SCALE)
                    es.append(e)
                for h in range(HEADS):
                    nc.tensor.matmul(
                        out=ctxp[h][:],
                        lhsT=vext[kc][:, h * (DH + 1):(h + 1) * (DH + 1)],
                        rhs=es[h][:], start=(kc == 0), stop=(kc == NCH - 1))

            ph5q.__exit__(None, None, None)
            ph5b = tc.tile_pool(name="bc_ps", bufs=1, space="PSUM")
            bc_ps = ph5b.__enter__()
            ctxT = [singles.tile([128, RPC], BF, name=f"ctxT{j}") for j in range(HCH)]
            ones1 = singles.tile([1, DH], F32)
            nc.vector.memset(ones1[:], 1.0)
            for h in range(HEADS):
                rs = singles.tile([1, RPC], F32, name=f"rs{h}", tag="rs")
                nc.vector.tensor_copy(out=rs[:], in_=ctxp[h][DH:DH + 1, :])
                rrec = singles.tile([1, RPC], F32, name=f"rrec{h}", tag="rrec")
                nc.vector.reciprocal(out=rrec[:], in_=rs[:])
                bc = bc_ps.tile([DH, RPC], F32, name=f"bc{h}", tag="bc")
                nc.tensor.matmul(out=bc[:], lhsT=ones1[:], rhs=rrec[:], start=True, stop=True)
                cs = singles.tile([DH, RPC], F32, name=f"cs{h}", tag="cs")
                nc.vector.tensor_copy(out=cs[:], in_=bc[:])
                j, r = h // 2, (h % 2) * 64
                nc.vector.tensor_tensor(out=ctxT[j][r:r + 64, :], in0=ctxp[h][0:DH, :],
                                        in1=cs[:], op=mybir.AluOpType.mult)

            ph5b.__exit__(None, None, None)
            ph5c.__exit__(None, None, None)
            ph6 = tc.tile_pool(name="fin_ps", bufs=2, space="PSUM")
            mid_ps = ph6.__enter__()

            # attn_out rows + residual + LN1
            xrows = [singles.tile([128, H], F32, name=f"xrows{i}") for i in range(NBLK)]
            for i in range(NBLK):
                ps = mid_ps.tile([128, H], F32)
                for k in range(HCH):
                    nc.tensor.matmul(out=ps[:], lhsT=ctxT[k][:, i * 128:(i + 1) * 128],
                                     rhs=Wo[k][:], start=(k == 0), stop=(k == HCH - 1))
                z = singles.tile([128, H], F32, name=f"z{i}", tag="zrow")
                nc.vector.tensor_add(z[:], ps[:], bob[:])
                nc.vector.tensor_add(z[:], z[:], hrows[i][:])
                _layernorm_rows(nc, singles, z, xrows[i], g1b, be1b, i, "ln1", epst)
            xT = [singles.tile([128, RPC], BF, name=f"xT{j}") for j in range(HCH)]
            for i in range(NBLK):
                for j in range(HCH):
                    tp = mid_ps.tile([128, 128], F32, name=f"tpx_{i}_{j}", tag="tp")
                    nc.tensor.transpose(out=tp[:], in_=xrows[i][:, j * 128:(j + 1) * 128],
                                        identity=ident[:])
                    nc.vector.tensor_copy(out=xT[j][:, i * 128:(i + 1) * 128], in_=tp[:])

            # FFN + LN2
            y1T = [singles.tile([128, RPC], BF, name=f"y1T{j}") for j in range(4)]
            for j in range(4):
                ps = mid_ps.tile([128, RPC], F32)
                for k in range(HCH):
                    nc.tensor.matmul(out=ps[:], lhsT=W1[k][:, j * 128:(j + 1) * 128],
                                     rhs=xT[k][:], start=(k == 0), stop=(k == HCH - 1))
                nc.scalar.activation(out=y1T[j][:], in_=ps[:], func=AF.Gelu,
                                     bias=b1c[:, j:j + 1], scale=1.0)
            for i in range(NBLK):
                ps = mid_ps.tile([128, H], F32)
                for k in range(4):
                    nc.tensor.matmul(out=ps[:], lhsT=y1T[k][:, i * 128:(i + 1) * 128],
                                     rhs=W2[k][:], start=(k == 0), stop=(k == 3))
                z = singles.tile([128, H], F32, name=f"z2{i}", tag="z2row")
                nc.vector.tensor_add(z[:], ps[:], b2b[:])
                nc.vector.tensor_add(z[:], z[:], xrows[i][:])
                o = singles.tile([128, H], F32, name=f"o{i}", tag="orow")
                _layernorm_rows(nc, singles, z, o, g2b, be2b, i, "ln2", epst)
                nc.sync.dma_start(out=out[i * 128:(i + 1) * 128, :], in_=o[:])
            ph6.__exit__(None, None, None)

    nc.compile()
    return nc


def _prep_edges(edge_index):
    src = np.asarray(edge_index[0]).astype(np.int64)
    dst = np.asarray(edge_index[1]).astype(np.int64)
    order = np.argsort(dst, kind="stable")
    src_s = src[order].astype(np.int32)
    dst_s = dst[order].astype(np.int32)
    blk = (dst_s // 128).astype(np.int64)
    counts = np.bincount(blk, minlength=N // 128)
    assert counts.max() <= TPB * 128, f"dst block overflow: {counts.max()} > {TPB*128}"
    starts = np.concatenate([[0], np.cumsum(counts)])
    per_core = []
    for c in range(NCORES):
        sT = np.full((NT * 128,), N, np.int32)
        dT = np.zeros((NT * 128,), np.int32)
        for b in range(NBLK):
            gb = c * NBLK + b
            s0, s1 = starts[gb], starts[gb + 1]
            n = s1 - s0
            o = b * TPB * 128
            sT[o:o + n] = src_s[s0:s1]
            dT[o:o + n] = dst_s[s0:s1] - c * RPC
            dT[o + n:o + TPB * 128] = b * 128
        per_core.append((np.ascontiguousarray(sT.reshape(NT, 128).T),
                         np.ascontiguousarray(dT.reshape(NT, 128).T)))
    return per_core


def kernel(**inputs):
    h = np.asarray(inputs["h"], np.float32)
    if "prog" not in _CACHE:
        _CACHE["prog"] = _build_program()
    nc = _CACHE["prog"]

    bf = ml_dtypes.bfloat16
    hT = np.ascontiguousarray(h.T).astype(bf)
    per_core_edges = _prep_edges(inputs["edge_index"])

    W = {k: np.asarray(inputs[k], np.float32) for k in
         ("W_src", "W_tgt", "Wq", "Wk", "Wv", "Wo", "W1", "W2")}
    B = {k: np.asarray(inputs[k], np.float32) for k in
         ("b_src", "b_tgt", "bq", "bk", "bv", "bo", "b1", "b2", "g1", "be1", "g2", "be2")}

    def bcast(v):
        return np.ascontiguousarray(np.tile(v[None, :], (128, 1)).astype(np.float32))

    common = {
        "hT_bf": hT,
        "w_src": W["W_src"].astype(bf), "w_tgt": W["W_tgt"].astype(bf),
        "w_q": W["Wq"].astype(bf), "w_k": W["Wk"].astype(bf),
        "w_v": W["Wv"].astype(bf), "w_o": W["Wo"].astype(bf),
        "w_1": W["W1"].astype(bf), "w_2": W["W2"].astype(bf),
        "bst_b": bcast(B["b_src"] + B["b_tgt"]),
        "bq_c": np.ascontiguousarray(B["bq"].reshape(HCH, 128).T.astype(np.float32)),
        "bk_c": np.ascontiguousarray(B["bk"].reshape(HCH, 128).T.astype(np.float32)),
        "bv_b": bcast(B["bv"]),
        "bo_b": bcast(B["bo"]),
        "b1_c": np.ascontiguousarray(B["b1"].reshape(4, 128).T.astype(np.float32)),
        "b2_b": bcast(B["b2"]),
        "g1_b": bcast(B["g1"]),
        "be1_b": bcast(B["be1"]),
        "g2_b": bcast(B["g2"]),
        "be2_b": bcast(B["be2"]),
    }
    in_maps = []
    for c in range(NCORES):
        sT, dT = per_core_edges[c]
        m = dict(common)
        m["srcT"] = sT
        m["dstT"] = dT
        m["h_rows"] = np.ascontiguousarray(h[c * RPC:(c + 1) * RPC, :])
        m["hT_own"] = np.ascontiguousarray(hT[:, c * RPC:(c + 1) * RPC])
        in_maps.append(m)

    res = run_bass_kernel_spmd(nc, in_maps, list(range(NCORES)))
    return np.concatenate([res.results[c]["out"] for c in range(NCORES)], axis=0)


if __name__ == "__main__":
    import reference
    inp = reference.setup_inputs()
    outp = kernel(**{k: np.asarray(v) for k, v in inp.items()})
    print("kernel out:", outp.shape, outp.dtype)
